# revision 14
# baseline (speedup 1.0000x reference)
"""Bass/Tile TRN2 kernel for nn_CRF_78907139162441 (CRF message passing), v6.

Math (per batch b, N=64 nodes, D=64*32*32=65536 features):
  F      = a_inter[b].reshape(N, D)
  G      = F @ F.T
  P      = G / (n_i n_j + 1e-6) * (W + W.T)/2
  e_0    = 0
  e_k[i] = sum_j tanh((u_i + e_{k-1}[j]) / 2) * P[i, j]   (10 iterations)
  out[b] = u + mean(e_10)

Sharding: pure data parallel, one batch per NeuronCore (8 cores).

v6 (HW-trace driven, see v4/v5 traces):
  - q-bands over d = p*512 + q: [384-wide (1.5 KiB DMA descriptors,
    ~365 GB/s, split into two node-half DMAs), then 128-wide last
    (512 B descs, ~270 GB/s) so only 4 MiB of work trails the stream].
  - f32->bf16 cast with the v4-form 3-dim APs (measured DVE 0.95 ns/el,
    Act ~2.1) - the 4-dim permuted form ran 2.7x slower.  DVE/Act
    chunk split ~70/30; GpSimd (5 ns/el) only takes small slices in
    the wide band where there is slack.
  - bf16 chunk layout (q-major, i-minor): matmul operands are plain
    contiguous [128,128]; 64 accumulating pair-matmuls per 128 q.
  - all DMA-independent prep (u/2, ubh, W+W.T) hoisted before the band
    loop so the iteration-1 tanh is not blocked behind the cast queue.
  - Act runs only {tanh, copy} (single act-table load via early junk
    tanh); DVE Newton rsqrt (bit-trick seed); scalar_tensor_tensor
    with accum_out fuses (q*P + row-reduce) on odd iterations.
"""

import os
import sys

import numpy as np

for _p in ("/opt/trn_rl_repo", "/root/.axon_site/_ro/trn_rl_repo"):
    if os.path.isdir(_p) and _p not in sys.path:
        sys.path.insert(0, _p)

import concourse.bass as bass
import concourse.bacc as bacc
import concourse.mybir as mybir
import concourse.tile as tile
from concourse.bass_utils import run_bass_kernel_spmd

B = 8          # batch / cores
N = 64         # nodes
D = 65536      # features per node
# (q0, width, n_dma_splits, chunk_pattern)
# 192-wide bands move with 768 B descriptors (~330 GB/s vs 270 at 512 B)
# and are castable the moment each single DMA lands; the 128-wide last
# band keeps the post-stream tail at 4 MiB.
BANDS = [
    # 1.5 KiB descriptors (measured ~339 GB/s); node-half-split DMAs with
    # per-half casts so the first half is cast while the second streams
    (0, 384, 2, [(22, "v"), (10, "s")]),
    # last band small so only ~2.8 us of casts trail the stream
    (384, 128, 2, [(22, "v"), (10, "s")]),
]
ITERATION = 10
NEWTON_ITERS = 1
MAGIC = 0x5F3759DF

F32 = mybir.dt.float32
BF16 = mybir.dt.bfloat16
I32 = mybir.dt.int32

_CACHE = {}

Alu = mybir.AluOpType


def build_nc():
    nc = bacc.Bacc("TRN2", target_bir_lowering=False, debug=False)

    a = nc.dram_tensor("a", [N, D], F32, kind="ExternalInput").ap()
    logits = nc.dram_tensor("logits", [N], F32, kind="ExternalInput").ap()
    w = nc.dram_tensor("w", [N, N], F32, kind="ExternalInput").ap()
    eye = nc.dram_tensor("eye", [128, 128], F32, kind="ExternalInput").ap()
    out = nc.dram_tensor("out", [N], F32, kind="ExternalOutput").ap()

    n_pairs = sum(bd[1] for bd in BANDS) // 2

    with tile.TileContext(nc) as tc:
        with (
            tc.tile_pool(name="io", bufs=1) as io,
            tc.tile_pool(name="bfp", bufs=1) as bfp,
            tc.tile_pool(name="small", bufs=1) as sm,
            tc.tile_pool(name="ps_g", bufs=1, space=bass.MemorySpace.PSUM) as ps_g,
            tc.tile_pool(name="ps_s", bufs=2, space=bass.MemorySpace.PSUM) as ps_s,
        ):
            # ---- constants ----
            ones_col = sm.tile([N, 1], F32)
            nc.vector.memset(ones_col[:], 1.0)
            ones_row = sm.tile([1, N], F32)
            nc.vector.memset(ones_row[:], 1.0)
            ones_nn = sm.tile([N, N], F32)
            nc.vector.memset(ones_nn[:], 1.0)

            eye_f = sm.tile([128, 128], F32)
            w_sb = sm.tile([N, N], F32)
            u_row = sm.tile([1, N], F32)
            u_col = sm.tile([N, 1], F32)

            # junk tanh: forces the (copy+tanh) act table load early
            junk = sm.tile([1, 1], F32)
            nc.scalar.activation(
                junk[:], ones_row[0:1, 0:1],
                mybir.ActivationFunctionType.Tanh,
            )

            # small inputs on the scalar queue
            nc.scalar.dma_start(eye_f[:], eye[:])
            nc.scalar.dma_start(w_sb[:], w[:])
            nc.scalar.dma_start(u_row[:], logits.rearrange("(o x) -> o x", o=1))
            nc.scalar.dma_start(u_col[:], logits.rearrange("(x o) -> x o", o=1))

            # ---- DMA-independent prep, hoisted before the stream ----
            u_half_col = sm.tile([N, 1], F32)
            nc.vector.tensor_scalar_mul(u_half_col[:], u_col[:], 0.5)
            u_half_row = sm.tile([1, N], F32)
            nc.vector.tensor_scalar_mul(u_half_row[:], u_row[:], 0.5)

            wt_ps = ps_s.tile([N, N], F32, tag="ps_small")
            nc.tensor.transpose(wt_ps[:], w_sb[:], eye_f[0:N, 0:N])
            ubh_ps = ps_s.tile([N, N], F32, tag="ps_small")
            nc.tensor.matmul(ubh_ps[:], ones_row[:], u_half_row[:])
            ubh = sm.tile([N, N], F32)
            nc.vector.tensor_copy(ubh[:], ubh_ps[:])

            wq = sm.tile([N, N], F32)  # (W + W.T)/4  [P/2 scaling]
            wsum = sm.tile([N, N], F32)
            nc.vector.tensor_add(wsum[:], w_sb[:], wt_ps[:])
            nc.vector.tensor_scalar_mul(wq[:], wsum[:], 0.25)

            # tanh(u/2) column, ready long before the tail needs it
            tu_col = sm.tile([N, 1], F32)
            nc.scalar.activation(
                tu_col[:], u_half_col[:],
                mybir.ActivationFunctionType.Tanh,
            )

            # ---- Gram via chunk-permuted stream: d = p*512 + q ----
            g_ps = ps_g.tile([128, 128], F32)
            av = a.rearrange("i (p q) -> p i q", p=128, q=512)
            H = N // 2
            (q0_a, wd_a, _, pat_a), (q0_b, wd_b, _, pat_b) = BANDS

            ftile_a = io.tile([128, N * wd_a], F32, tag="ftile0")
            ftile_b = io.tile([128, N * wd_b], F32, tag="ftile1")
            fa3 = ftile_a[:].rearrange("p (i q) -> p i q", q=wd_a)
            fb3 = ftile_b[:].rearrange("p (i q) -> p i q", q=wd_b)
            # DMA order: small-band first half first, so the fabric's
            # first-transfer warm-up penalty lands on 2 MiB, not 6 MiB;
            # small-band second half last keeps the tail at 2 MiB of casts
            nc.sync.dma_start(fb3[:, 0:H, :], av[:, 0:H, q0_b : q0_b + wd_b])
            nc.sync.dma_start(fa3[:, 0:H, :], av[:, 0:H, q0_a : q0_a + wd_a])
            nc.sync.dma_start(fa3[:, H:, :], av[:, H:, q0_a : q0_a + wd_a])
            nc.sync.dma_start(fb3[:, H:, :], av[:, H:, q0_b : q0_b + wd_b])

            def alloc_chunks(bi, wd, pat, ftile):
                fview = ftile[:].rearrange("p (i q) -> p q i", q=wd)
                chunks = []
                cq = 0
                psz = sum(cw for cw, _ in pat)
                for rep in range(wd // psz):
                    for pi, (cw, ceng) in enumerate(pat):
                        btc = bfp.tile(
                            [128, N * cw], BF16, tag=f"b{bi}c{pi}_{rep}"
                        )
                        btc_v = btc[:].rearrange("p (q i) -> p q i", i=N)
                        chunks.append(
                            (btc, btc_v, fview[:, cq : cq + cw, :], cw, ceng)
                        )
                        cq += cw
                return chunks

            def half_casts(chunks, lo, hi):
                for btc, btc_v, src, cw, ceng in chunks:
                    if ceng == "v":
                        nc.vector.tensor_copy(
                            btc_v[:, :, lo:hi], src[:, :, lo:hi]
                        )
                    else:
                        nc.scalar.copy(btc_v[:, :, lo:hi], src[:, :, lo:hi])

            chunks_a = alloc_chunks(0, wd_a, pat_a, ftile_a)
            chunks_b = alloc_chunks(1, wd_b, pat_b, ftile_b)

            # cast phases in data-arrival order
            half_casts(chunks_b, 0, H)
            half_casts(chunks_a, 0, H)

            k = 0
            for btc, btc_v, src, cw, ceng in chunks_a:
                if ceng == "v":
                    nc.vector.tensor_copy(btc_v[:, :, H:], src[:, :, H:])
                else:
                    nc.scalar.copy(btc_v[:, :, H:], src[:, :, H:])
                for u in range(cw // 2):
                    mm = btc[:, u * 128 : (u + 1) * 128]
                    nc.tensor.matmul(
                        g_ps[:], mm, mm,
                        start=(k == 0), stop=(k == n_pairs - 1),
                    )
                    k += 1
            for btc, btc_v, src, cw, ceng in chunks_b:
                if ceng == "v":
                    nc.vector.tensor_copy(btc_v[:, :, H:], src[:, :, H:])
                else:
                    nc.scalar.copy(btc_v[:, :, H:], src[:, :, H:])
                for u in range(cw // 2):
                    mm = btc[:, u * 128 : (u + 1) * 128]
                    nc.tensor.matmul(
                        g_ps[:], mm, mm,
                        start=(k == 0), stop=(k == n_pairs - 1),
                    )
                    k += 1

            # ---- G = even-q diag block + odd-q diag block ----
            g_sb = sm.tile([N, N], F32)
            g_hi = sm.tile([N, N], F32)
            nc.vector.tensor_copy(g_hi[:], g_ps[N : 2 * N, N : 2 * N])
            nc.vector.tensor_add(g_sb[:], g_ps[0:N, 0:N], g_hi[:])

            # ---- rsq = 1/sqrt(diag(G)) as a column, on DVE only ----
            gi = sm.tile([N, N], F32)
            nc.vector.tensor_mul(gi[:], g_sb[:], eye_f[0:N, 0:N])
            n2c = sm.tile([N, 1], F32)
            nc.vector.tensor_reduce(
                n2c[:], gi[:], mybir.AxisListType.X, Alu.add
            )
            m1 = sm.tile([N, N], F32)
            nc.vector.tensor_mul(m1[:], g_sb[:], wq[:])

            ya = sm.tile([N, 1], F32)
            yb = sm.tile([N, 1], F32)
            t1 = sm.tile([N, 1], F32)
            nc.vector.tensor_scalar(
                t1[:].bitcast(I32), n2c[:].bitcast(I32), 1, -1,
                op0=Alu.logical_shift_right, op1=Alu.bitwise_xor,
            )
            nc.vector.tensor_scalar(
                ya[:].bitcast(I32), t1[:].bitcast(I32), MAGIC + 1, None,
                op0=Alu.add,
            )
            cur = ya
            for nit in range(NEWTON_ITERS):
                dst = yb if cur is ya else ya
                nc.vector.tensor_mul(t1[:], n2c[:], cur[:])
                nc.vector.tensor_mul(t1[:], t1[:], cur[:])
                nc.vector.tensor_scalar(
                    t1[:], t1[:], -0.5, 1.5, op0=Alu.mult, op1=Alu.add
                )
                nc.vector.tensor_mul(dst[:], t1[:], cur[:])
                cur = dst
            rsq_col = cur

            # iteration 1 collapses (e_0 = 0 makes tanh rank-1):
            #   h_1 = tanh(u/2) * rsq * (m1 @ rsq)
            mv_ps = ps_s.tile([N, 1], F32, tag="ps_small")
            nc.tensor.matmul(mv_ps[:], m1[:], rsq_col[:])
            mv_sb = sm.tile([N, 1], F32)
            nc.vector.tensor_copy(mv_sb[:], mv_ps[:])
            h_col = sm.tile([N, 1], F32)
            nc.vector.scalar_tensor_tensor(
                h_col[:], tu_col[:], rsq_col[:], mv_sb[:],
                op0=Alu.mult, op1=Alu.mult,
            )

            # P/2 for iterations 2..10 (overlaps the iter-2 tanh):
            # rsq row via PE transpose, outer product, then m1 * outer
            rr_ps = ps_s.tile([1, N], F32, tag="ps_small")
            nc.tensor.matmul(rr_ps[:], rsq_col[:], eye_f[0:N, 0:N])
            rsq_row = sm.tile([1, N], F32)
            nc.vector.tensor_copy(rsq_row[:], rr_ps[:])
            outer_ps = ps_s.tile([N, N], F32, tag="ps_small")
            nc.tensor.matmul(outer_ps[:], rsq_row[:], rsq_row[:])
            p_sb = sm.tile([N, N], F32)
            nc.vector.tensor_mul(p_sb[:], m1[:], outer_ps[:])

            # ---- iterations 2..10, state h = e/2 ----
            q_sb = sm.tile([N, N], F32)
            qp = sm.tile([N, N], F32)
            hfr_src = None
            for it in range(2, ITERATION + 1):
                if it % 2 == 1:
                    nc.scalar.activation(
                        q_sb[:], hfr_src,
                        mybir.ActivationFunctionType.Tanh,
                        bias=u_half_col[:],
                    )
                    nc.vector.scalar_tensor_tensor(
                        qp[:], q_sb[:], 1.0, p_sb[:],
                        op0=Alu.mult, op1=Alu.mult,
                        accum_out=h_col[:],
                    )
                else:
                    nc.scalar.activation(
                        q_sb[:], ubh[:],
                        mybir.ActivationFunctionType.Tanh,
                        bias=h_col[:],
                    )
                    nc.vector.tensor_mul(qp[:], q_sb[:], p_sb[:])
                    if it == ITERATION:
                        # last iteration: only the summary row is needed
                        row_ps = ps_s.tile([1, N], F32, tag="ps_small")
                        nc.tensor.matmul(row_ps[:], ones_col[:], qp[:])
                        hfr_src = None
                    else:
                        hfr_ps = ps_s.tile([N, N], F32, tag="ps_small")
                        nc.tensor.matmul(hfr_ps[:], ones_nn[:], qp[:])
                        hfr_src = hfr_ps[:]

            # ---- out = u + (2/N) * sum_j e_10[j] ----
            red = sm.tile([1, 1], F32)
            nc.vector.tensor_reduce(
                red[:], row_ps[0:1, :], mybir.AxisListType.X, Alu.add
            )
            red2 = sm.tile([1, 1], F32)
            nc.vector.tensor_scalar_mul(red2[:], red[:], 2.0 / N)
            out_sb = sm.tile([1, N], F32)
            nc.vector.tensor_scalar(
                out_sb[:], u_row[:], red2[:], None, op0=Alu.add
            )
            nc.gpsimd.dma_start(out.rearrange("(o x) -> o x", o=1), out_sb[:])

    nc.compile()
    return nc


def _in_maps(inputs):
    a_inter = np.ascontiguousarray(inputs["a_inter"], dtype=np.float32)
    logits = np.ascontiguousarray(inputs["logits"], dtype=np.float32)
    w = np.ascontiguousarray(inputs["W"], dtype=np.float32)[0]
    eye = np.eye(128, dtype=np.float32)
    return [
        {
            "a": a_inter[b].reshape(N, D).copy(),
            "logits": logits[b].copy(),
            "w": w.copy(),
            "eye": eye,
        }
        for b in range(B)
    ]


def kernel(**inputs) -> np.ndarray:
    if "nc" not in _CACHE:
        _CACHE["nc"] = build_nc()
    nc = _CACHE["nc"]
    res = run_bass_kernel_spmd(nc, _in_maps(inputs), core_ids=list(range(B)))
    return np.stack([res.results[b]["out"] for b in range(B)], axis=0)


if __name__ == "__main__":
    rng = np.random.default_rng(0)
    ins = {
        "a_inter": rng.standard_normal((B, N, N, 32, 32), dtype=np.float32),
        "logits": rng.standard_normal((B, N), dtype=np.float32),
        "W": rng.standard_normal((1, N, N), dtype=np.float32),
    }
    print(kernel(**ins).shape)


# revision 16
# speedup vs baseline: 1.0165x; 1.0165x over previous
"""Bass/Tile TRN2 kernel for nn_CRF_78907139162441 (CRF message passing), v6.

Math (per batch b, N=64 nodes, D=64*32*32=65536 features):
  F      = a_inter[b].reshape(N, D)
  G      = F @ F.T
  P      = G / (n_i n_j + 1e-6) * (W + W.T)/2
  e_0    = 0
  e_k[i] = sum_j tanh((u_i + e_{k-1}[j]) / 2) * P[i, j]   (10 iterations)
  out[b] = u + mean(e_10)

Sharding: pure data parallel, one batch per NeuronCore (8 cores).

v6 (HW-trace driven, see v4/v5 traces):
  - q-bands over d = p*512 + q: [384-wide (1.5 KiB DMA descriptors,
    ~365 GB/s, split into two node-half DMAs), then 128-wide last
    (512 B descs, ~270 GB/s) so only 4 MiB of work trails the stream].
  - f32->bf16 cast with the v4-form 3-dim APs (measured DVE 0.95 ns/el,
    Act ~2.1) - the 4-dim permuted form ran 2.7x slower.  DVE/Act
    chunk split ~70/30; GpSimd (5 ns/el) only takes small slices in
    the wide band where there is slack.
  - bf16 chunk layout (q-major, i-minor): matmul operands are plain
    contiguous [128,128]; 64 accumulating pair-matmuls per 128 q.
  - all DMA-independent prep (u/2, ubh, W+W.T) hoisted before the band
    loop so the iteration-1 tanh is not blocked behind the cast queue.
  - Act runs only {tanh, copy} (single act-table load via early junk
    tanh); DVE Newton rsqrt (bit-trick seed); scalar_tensor_tensor
    with accum_out fuses (q*P + row-reduce) on odd iterations.
"""

import os
import sys

import numpy as np

for _p in ("/opt/trn_rl_repo", "/root/.axon_site/_ro/trn_rl_repo"):
    if os.path.isdir(_p) and _p not in sys.path:
        sys.path.insert(0, _p)

import concourse.bass as bass
import concourse.bacc as bacc
import concourse.mybir as mybir
import concourse.tile as tile
from concourse.bass_utils import run_bass_kernel_spmd

B = 8          # batch / cores
N = 64         # nodes
D = 65536      # features per node
# (q0, width, n_dma_splits, chunk_pattern)
# 192-wide bands move with 768 B descriptors (~330 GB/s vs 270 at 512 B)
# and are castable the moment each single DMA lands; the 128-wide last
# band keeps the post-stream tail at 4 MiB.
BANDS = [
    # 1.5 KiB descriptors (measured ~339 GB/s); node-half-split DMAs with
    # per-half casts so the first half is cast while the second streams
    (0, 384, 2, [(22, "v"), (10, "s")]),
    # last band small so only ~2.8 us of casts trail the stream
    (384, 128, 2, [(22, "v"), (10, "s")]),
]
ITERATION = 10
NEWTON_ITERS = 1
MAGIC = 0x5F3759DF

F32 = mybir.dt.float32
BF16 = mybir.dt.bfloat16
I32 = mybir.dt.int32

_CACHE = {}

Alu = mybir.AluOpType


def build_nc():
    nc = bacc.Bacc("TRN2", target_bir_lowering=False, debug=False)

    a = nc.dram_tensor("a", [N, D], F32, kind="ExternalInput").ap()
    logits = nc.dram_tensor("logits", [N], F32, kind="ExternalInput").ap()
    w = nc.dram_tensor("w", [N, N], F32, kind="ExternalInput").ap()
    eye = nc.dram_tensor("eye", [128, 128], F32, kind="ExternalInput").ap()
    out = nc.dram_tensor("out", [N], F32, kind="ExternalOutput").ap()

    n_pairs = sum(bd[1] for bd in BANDS) // 2

    with tile.TileContext(nc) as tc:
        with (
            tc.tile_pool(name="io", bufs=1) as io,
            tc.tile_pool(name="bfp", bufs=1) as bfp,
            tc.tile_pool(name="small", bufs=1) as sm,
            tc.tile_pool(name="ps_g", bufs=1, space=bass.MemorySpace.PSUM) as ps_g,
            tc.tile_pool(name="ps_s", bufs=2, space=bass.MemorySpace.PSUM) as ps_s,
        ):
            # ---- constants ----
            ones_col = sm.tile([N, 1], F32)
            nc.vector.memset(ones_col[:], 1.0)
            ones_row = sm.tile([1, N], F32)
            nc.vector.memset(ones_row[:], 1.0)
            ones_nn = sm.tile([N, N], F32)
            nc.vector.memset(ones_nn[:], 1.0)

            eye_f = sm.tile([128, 128], F32)
            w_sb = sm.tile([N, N], F32)
            u_row = sm.tile([1, N], F32)
            u_col = sm.tile([N, 1], F32)

            # junk tanh: forces the (copy+tanh) act table load early
            junk = sm.tile([1, 1], F32)
            nc.scalar.activation(
                junk[:], ones_row[0:1, 0:1],
                mybir.ActivationFunctionType.Tanh,
            )

            # small inputs on the scalar queue
            nc.scalar.dma_start(eye_f[:], eye[:])
            nc.scalar.dma_start(w_sb[:], w[:])
            nc.scalar.dma_start(u_row[:], logits.rearrange("(o x) -> o x", o=1))
            nc.scalar.dma_start(u_col[:], logits.rearrange("(x o) -> x o", o=1))

            # ---- DMA-independent prep, hoisted before the stream ----
            u_half_col = sm.tile([N, 1], F32)
            nc.vector.tensor_scalar_mul(u_half_col[:], u_col[:], 0.5)
            u_half_row = sm.tile([1, N], F32)
            nc.vector.tensor_scalar_mul(u_half_row[:], u_row[:], 0.5)

            wt_ps = ps_s.tile([N, N], F32, tag="ps_small")
            nc.tensor.transpose(wt_ps[:], w_sb[:], eye_f[0:N, 0:N])
            ubh_ps = ps_s.tile([N, N], F32, tag="ps_small")
            nc.tensor.matmul(ubh_ps[:], ones_row[:], u_half_row[:])
            ubh = sm.tile([N, N], F32)
            nc.vector.tensor_copy(ubh[:], ubh_ps[:])

            wq = sm.tile([N, N], F32)  # (W + W.T)/4  [P/2 scaling]
            wsum = sm.tile([N, N], F32)
            nc.vector.tensor_add(wsum[:], w_sb[:], wt_ps[:])
            nc.vector.tensor_scalar_mul(wq[:], wsum[:], 0.25)

            # tanh(u/2) column, ready long before the tail needs it
            tu_col = sm.tile([N, 1], F32)
            nc.scalar.activation(
                tu_col[:], u_half_col[:],
                mybir.ActivationFunctionType.Tanh,
            )

            # ---- Gram via chunk-permuted stream: d = p*512 + q ----
            g_ps = ps_g.tile([128, 128], F32)
            av = a.rearrange("i (p q) -> p i q", p=128, q=512)
            k = 0
            for bi, (q0, wd, nsplit, pat) in enumerate(BANDS):
                ftile = io.tile([128, N * wd], F32, tag=f"ftile{bi}")
                fv3 = ftile[:].rearrange("p (i q) -> p i q", q=wd)
                istep = N // nsplit
                for si in range(nsplit):
                    nc.sync.dma_start(
                        fv3[:, si * istep : (si + 1) * istep, :],
                        av[:, si * istep : (si + 1) * istep, q0 : q0 + wd],
                    )
                fview = ftile[:].rearrange("p (i q) -> p q i", q=wd)
                psum = sum(cw for cw, _ in pat)
                if nsplit == 2:
                    # split-cast band: cast each node half as its DMA lands,
                    # so only the second half's casts trail the stream
                    H = N // 2
                    chunks = []
                    cq = 0
                    for rep in range(wd // psum):
                        for pi, (cw, ceng) in enumerate(pat):
                            btc = bfp.tile(
                                [128, N * cw], BF16, tag=f"b{bi}c{pi}_{rep}"
                            )
                            btc_v = btc[:].rearrange("p (q i) -> p q i", i=N)
                            src = fview[:, cq : cq + cw, :]
                            if ceng == "v":
                                nc.vector.tensor_copy(
                                    btc_v[:, :, 0:H], src[:, :, 0:H]
                                )
                            else:
                                nc.scalar.copy(
                                    btc_v[:, :, 0:H], src[:, :, 0:H]
                                )
                            chunks.append((btc, btc_v, src, cw, ceng))
                            cq += cw
                    if bi == 0:
                        warm_tile = chunks[0][0]
                    if bi == len(BANDS) - 1:
                        # keep the PE clock up through its idle window so
                        # the last band's matmuls issue at warm-clock rate;
                        # sized to finish before the real operands can be
                        # ready, so they can never delay real work
                        warm_mm = warm_tile[:, 0:128]
                        warm_ps = ps_s.tile([128, 128], F32, tag="ps_warm")
                        for _ in range(40):
                            nc.tensor.matmul(
                                warm_ps[:], warm_mm, warm_mm,
                                start=True, stop=True,
                            )
                    for btc, btc_v, src, cw, ceng in chunks:
                        if ceng == "v":
                            nc.vector.tensor_copy(
                                btc_v[:, :, H:], src[:, :, H:]
                            )
                        else:
                            nc.scalar.copy(btc_v[:, :, H:], src[:, :, H:])
                        for u in range(cw // 2):
                            mm = btc[:, u * 128 : (u + 1) * 128]
                            nc.tensor.matmul(
                                g_ps[:],
                                mm,
                                mm,
                                start=(k == 0),
                                stop=(k == n_pairs - 1),
                            )
                            k += 1
                else:
                    cq = 0
                    for rep in range(wd // psum):
                        for pi, (cw, ceng) in enumerate(pat):
                            U = cw // 2
                            btc = bfp.tile(
                                [128, N * cw], BF16, tag=f"b{bi}c{pi}_{rep % 2}"
                            )
                            btc_v = btc[:].rearrange("p (q i) -> p q i", i=N)
                            src = fview[:, cq : cq + cw, :]
                            if ceng == "v":
                                nc.vector.tensor_copy(btc_v, src)
                            elif ceng == "s":
                                nc.scalar.copy(btc_v, src)
                            else:
                                nc.gpsimd.tensor_copy(btc_v, src)
                            for u in range(U):
                                mm = btc[:, u * 128 : (u + 1) * 128]
                                nc.tensor.matmul(
                                    g_ps[:],
                                    mm,
                                    mm,
                                    start=(k == 0),
                                    stop=(k == n_pairs - 1),
                                )
                                k += 1
                            cq += cw

            # ---- G = even-q diag block + odd-q diag block ----
            g_sb = sm.tile([N, N], F32)
            g_hi = sm.tile([N, N], F32)
            nc.vector.tensor_copy(g_hi[:], g_ps[N : 2 * N, N : 2 * N])
            nc.vector.tensor_add(g_sb[:], g_ps[0:N, 0:N], g_hi[:])

            # ---- rsq = 1/sqrt(diag(G)) as a column, on DVE only ----
            gi = sm.tile([N, N], F32)
            nc.vector.tensor_mul(gi[:], g_sb[:], eye_f[0:N, 0:N])
            n2c = sm.tile([N, 1], F32)
            nc.vector.tensor_reduce(
                n2c[:], gi[:], mybir.AxisListType.X, Alu.add
            )
            m1 = sm.tile([N, N], F32)
            nc.vector.tensor_mul(m1[:], g_sb[:], wq[:])

            ya = sm.tile([N, 1], F32)
            yb = sm.tile([N, 1], F32)
            t1 = sm.tile([N, 1], F32)
            nc.vector.tensor_scalar(
                t1[:].bitcast(I32), n2c[:].bitcast(I32), 1, -1,
                op0=Alu.logical_shift_right, op1=Alu.bitwise_xor,
            )
            nc.vector.tensor_scalar(
                ya[:].bitcast(I32), t1[:].bitcast(I32), MAGIC + 1, None,
                op0=Alu.add,
            )
            cur = ya
            for nit in range(NEWTON_ITERS):
                dst = yb if cur is ya else ya
                nc.vector.tensor_mul(t1[:], n2c[:], cur[:])
                nc.vector.tensor_mul(t1[:], t1[:], cur[:])
                nc.vector.tensor_scalar(
                    t1[:], t1[:], -0.5, 1.5, op0=Alu.mult, op1=Alu.add
                )
                nc.vector.tensor_mul(dst[:], t1[:], cur[:])
                cur = dst
            rsq_col = cur

            # iteration 1 collapses (e_0 = 0 makes tanh rank-1):
            #   h_1 = tanh(u/2) * rsq * (m1 @ rsq)
            mv_ps = ps_s.tile([N, 1], F32, tag="ps_small")
            nc.tensor.matmul(mv_ps[:], m1[:], rsq_col[:])
            mv_sb = sm.tile([N, 1], F32)
            nc.vector.tensor_copy(mv_sb[:], mv_ps[:])
            h_col = sm.tile([N, 1], F32)
            nc.vector.scalar_tensor_tensor(
                h_col[:], tu_col[:], rsq_col[:], mv_sb[:],
                op0=Alu.mult, op1=Alu.mult,
            )

            # P/2 for iterations 2..10 (overlaps the iter-2 tanh):
            # rsq row via PE transpose, outer product, then m1 * outer
            rr_ps = ps_s.tile([1, N], F32, tag="ps_small")
            nc.tensor.matmul(rr_ps[:], rsq_col[:], eye_f[0:N, 0:N])
            rsq_row = sm.tile([1, N], F32)
            nc.vector.tensor_copy(rsq_row[:], rr_ps[:])
            outer_ps = ps_s.tile([N, N], F32, tag="ps_small")
            nc.tensor.matmul(outer_ps[:], rsq_row[:], rsq_row[:])
            p_sb = sm.tile([N, N], F32)
            nc.vector.tensor_mul(p_sb[:], m1[:], outer_ps[:])

            # ---- iterations 2..10, state h = e/2 ----
            q_sb = sm.tile([N, N], F32)
            qp = sm.tile([N, N], F32)
            hfr_src = None
            for it in range(2, ITERATION + 1):
                if it % 2 == 1:
                    nc.scalar.activation(
                        q_sb[:], hfr_src,
                        mybir.ActivationFunctionType.Tanh,
                        bias=u_half_col[:],
                    )
                    nc.vector.scalar_tensor_tensor(
                        qp[:], q_sb[:], 1.0, p_sb[:],
                        op0=Alu.mult, op1=Alu.mult,
                        accum_out=h_col[:],
                    )
                else:
                    nc.scalar.activation(
                        q_sb[:], ubh[:],
                        mybir.ActivationFunctionType.Tanh,
                        bias=h_col[:],
                    )
                    nc.vector.tensor_mul(qp[:], q_sb[:], p_sb[:])
                    if it == ITERATION:
                        # last iteration: only the summary row is needed
                        row_ps = ps_s.tile([1, N], F32, tag="ps_small")
                        nc.tensor.matmul(row_ps[:], ones_col[:], qp[:])
                        hfr_src = None
                    else:
                        hfr_ps = ps_s.tile([N, N], F32, tag="ps_small")
                        nc.tensor.matmul(hfr_ps[:], ones_nn[:], qp[:])
                        hfr_src = hfr_ps[:]

            # ---- out = u + (2/N) * sum_j e_10[j] ----
            red = sm.tile([1, 1], F32)
            nc.vector.tensor_reduce(
                red[:], row_ps[0:1, :], mybir.AxisListType.X, Alu.add
            )
            red2 = sm.tile([1, 1], F32)
            nc.vector.tensor_scalar_mul(red2[:], red[:], 2.0 / N)
            out_sb = sm.tile([1, N], F32)
            nc.vector.tensor_scalar(
                out_sb[:], u_row[:], red2[:], None, op0=Alu.add
            )
            nc.gpsimd.dma_start(out.rearrange("(o x) -> o x", o=1), out_sb[:])

    nc.compile()
    return nc


def _in_maps(inputs):
    a_inter = np.ascontiguousarray(inputs["a_inter"], dtype=np.float32)
    logits = np.ascontiguousarray(inputs["logits"], dtype=np.float32)
    w = np.ascontiguousarray(inputs["W"], dtype=np.float32)[0]
    eye = np.eye(128, dtype=np.float32)
    return [
        {
            "a": a_inter[b].reshape(N, D).copy(),
            "logits": logits[b].copy(),
            "w": w.copy(),
            "eye": eye,
        }
        for b in range(B)
    ]


def kernel(**inputs) -> np.ndarray:
    if "nc" not in _CACHE:
        _CACHE["nc"] = build_nc()
    nc = _CACHE["nc"]
    res = run_bass_kernel_spmd(nc, _in_maps(inputs), core_ids=list(range(B)))
    return np.stack([res.results[b]["out"] for b in range(B)], axis=0)


if __name__ == "__main__":
    rng = np.random.default_rng(0)
    ins = {
        "a_inter": rng.standard_normal((B, N, N, 32, 32), dtype=np.float32),
        "logits": rng.standard_normal((B, N), dtype=np.float32),
        "W": rng.standard_normal((1, N, N), dtype=np.float32),
    }
    print(kernel(**ins).shape)
